# revision 23
# baseline (speedup 1.0000x reference)
"""Trainium2 Bass kernel for nn_BinaryAttention (dense transformer block).

Strategy: data-parallel over batch, 8 cores x 8 batch elements each.
v2: zero-gpsimd pipeline. Per core (4 chunks of 2 batches):
  QKV proj: Q/K in 3-pass fp16 split precision (sign() boundaries need
    ~22-bit products); V 1-pass fp16. ACT Sign -> fp8 sq/sk; DVE
    abs-reduce -> per-(b,h) scale partials.
  scales: batched per chunk via indicator matmul + one STT.
  V quantize: DVE TS (x*63.5 + 1536 -> fp16, RNE int round), DVE
    offset-clip [1409,1663], ACT Identity -1536 -> exact int8 grid fp16.
  attention (per (b,h) pair): S = sq@sk^T exact fp8 matmul; one merged
    ACT Exp [128,394] (scale=s_q*s_k/8 per-partition AP); DVE STT
    e = f*expb with fused rowsum accum; batched recip+*255; P-round in
    ONE op per half: fp16(e*rr + 1024) = 1024 + round(255*p) (RNE via
    fp16 convert); DMA-xbar transpose (both heads of an fc in one
    instruction); PV matmul in fp16 carries the +1024 offset, corrected
    exactly via per-partition ACT bias (-1024*CPV*colsum(V), colsums by
    tiny PE matmuls).
  proj: 1-pass fp16, bias added via ACT Identity on the PSUM->SBUF copy.
Chains software-pipelined one chunk behind their QK projection so
ACT/DVE chew on chunk i while PE runs chunk i+1 matmuls.
"""
import sys
sys.path.insert(0, "/opt/trn_rl_repo")
import os
import numpy as np
from contextlib import ExitStack

import concourse.bass as bass
import concourse.tile as tile
from concourse import bacc, mybir
from concourse.bass_utils import run_bass_kernel_spmd

F32 = mybir.dt.float32
F16 = mybir.dt.float16
BF16 = mybir.dt.bfloat16
F8 = mybir.dt.float8e4
AF = mybir.ActivationFunctionType
OP = mybir.AluOpType

NCORES = 8
B, N, C = 64, 197, 768
H, D = 12, 64
KC = C // 128               # 6 contraction chunks
BL = B // NCORES            # 8 batches per core
T = BL * N                  # 1576 tokens per core
TCH = 2 * N                 # 394-token chunks (2 batches), 4 per core
SCALE = float(D) ** -0.5
CPV = float(np.float32(1.0 / 255.0) * np.float32(2.0 / 127.0))
MP = 1024.0                 # P-round offset: p+MP in [1024,2048) = fp16 int grid
MV = 1536.0                 # V-round offset: v*63.5+MV in [1409,1663]
CORR = float(-MP * CPV)

_CACHE = {}


def _build():
    nc = bacc.Bacc(None, target_bir_lowering=False, debug=False)
    d = {}

    def din(name, shape, dt):
        d[name] = nc.dram_tensor(name, list(shape), dt, kind="ExternalInput")

    din("xh", (KC, 128, T), F16)
    din("xl", (KC, 128, T), F16)
    din("wqkh", (KC, 128, 2 * C), F16)
    din("wqkl", (KC, 128, 2 * C), F16)
    din("wv", (KC, 128, C), F16)
    din("wp", (KC, 128, C), F16)
    din("pbT", (128, KC), F32)          # proj bias, feature-partition layout
    din("eb1", (128, H, N), F32)        # exp(bias) rows n=0..127   [n, h, m]
    din("eb2", (69, H, N), F32)         # exp(bias) rows n=128..196
    din("ind2", (128, 256), F32)
    d["yt"] = nc.dram_tensor("yt", [C, T], F32, kind="ExternalOutput")
    DUMP = os.environ.get("DUMP_OT")
    if DUMP:
        d["otd"] = nc.dram_tensor("otd", [KC, 128, T], F16, kind="ExternalOutput")
        d["p1d"] = nc.dram_tensor("p1d", [128, 4, 256], F16, kind="ExternalOutput")
        d["p2d"] = nc.dram_tensor("p2d", [80, 4, 256], F16, kind="ExternalOutput")
        d["ttd"] = nc.dram_tensor("ttd", [128, 8, 208], F16, kind="ExternalOutput")
        d["vd"] = nc.dram_tensor("vd", [BL, 128, C], F16, kind="ExternalOutput")
        d["rd"] = nc.dram_tensor("rd", [128, 2, 96], F32, kind="ExternalOutput")

    with tile.TileContext(nc) as tc, ExitStack() as ctx:
        pers = ctx.enter_context(tc.tile_pool(name="pers", bufs=1))
        pw = ctx.enter_context(tc.tile_pool(name="pw", bufs=1))
        pxs = ctx.enter_context(tc.tile_pool(name="pxs", bufs=2))
        pvt = ctx.enter_context(tc.tile_pool(name="pvt", bufs=2))
        pwork = ctx.enter_context(tc.tile_pool(name="pwork", bufs=3))
        pe1 = ctx.enter_context(tc.tile_pool(name="pe1", bufs=6))
        ppt = ctx.enter_context(tc.tile_pool(name="ppt", bufs=2))
        ptt = ctx.enter_context(tc.tile_pool(name="ptt", bufs=4))
        # PSUM: exactly 8 banks
        pqk = ctx.enter_context(tc.tile_pool(name="pqk", bufs=2, space="PSUM"))
        ppv = ctx.enter_context(tc.tile_pool(name="ppv", bufs=2, space="PSUM"))
        pps2 = ctx.enter_context(tc.tile_pool(name="pps2", bufs=2, space="PSUM"))
        ppo = ctx.enter_context(tc.tile_pool(name="ppo", bufs=1, space="PSUM"))
        ppy = ctx.enter_context(tc.tile_pool(name="ppy", bufs=1, space="PSUM"))

        # ---- persistent state / constants (DMAs issued after the weight
        # loads on the scalar queue; sync queue stays free for x chunk 0)
        ind2 = pers.tile([128, 256], F32, tag="ind2")
        eb1s = pers.tile([128, H, N], F32, tag="eb1s")
        eb2s = pers.tile([69, H, N], F32, tag="eb2s")
        pbTs = pers.tile([128, KC], F32, tag="pbTs")

        onesc = pers.tile([128, 1], F16, tag="onesc")
        nc.vector.memset(onesc[:], 1.0)
        bMP = pers.tile([128, 1], F32, tag="bMP")
        nc.vector.memset(bMP[:], MP)
        bmMV = pers.tile([128, 1], F32, tag="bmMV")
        nc.vector.memset(bmMV[:], -MV)

        sq = [pers.tile([128, T], F8, tag=f"sq{i}", name=f"sq{i}") for i in range(KC)]
        sk = [pers.tile([128, T], F8, tag=f"sk{i}", name=f"sk{i}") for i in range(KC)]
        vint = [(pers.tile([128, C], F16, tag=f"v1_{b}", name=f"v1_{b}"),
                 pers.tile([69, C], F16, tag=f"v2_{b}", name=f"v2_{b}")) for b in range(BL)]
        ot = [pers.tile([128, T], F16, tag=f"ot{i}", name=f"ot{i}") for i in range(KC)]
        pa = pers.tile([128, 96], F32, tag="pa")
        sqkA = pers.tile([128, 48], F32, tag="sqkA")   # col = tci*12+fc*2+b2
        sqkB = pers.tile([128, 48], F32, tag="sqkB")
        rAB = pers.tile([128, 2, 96], F32, tag="rAB")  # col = b*12+fc*2+j
        rrAB = pers.tile([128, 2, 96], F32, tag="rrAB")
        rrS = pers.tile([128, 2, 96], F32, tag="rrS")
        vsb = pers.tile([128, BL, KC], F32, tag="vsb")

        # ---- weights resident
        wqht = pw.tile([128, KC, 2 * C], F16, tag="wqht")
        wqlt = pw.tile([128, KC, 2 * C], F16, tag="wqlt")
        wvt = pw.tile([128, KC, C], F16, tag="wvt")
        wpt = pw.tile([128, KC, C], F16, tag="wpt")
        # per-k loads in QK-use order so fcp0 starts after the first slices
        for k in range(KC):
            nc.scalar.dma_start(out=wqht[:, k], in_=d["wqkh"][k])
        for k in range(KC):
            nc.scalar.dma_start(out=wqlt[:, k], in_=d["wqkl"][k])
        for k in range(KC):
            nc.scalar.dma_start(out=wvt[:, k], in_=d["wv"][k])
        for k in range(KC):
            nc.scalar.dma_start(out=wpt[:, k], in_=d["wp"][k])
        nc.scalar.dma_start(out=ind2[:], in_=d["ind2"][:])
        nc.scalar.dma_start(out=eb1s[:], in_=d["eb1"][:])
        nc.scalar.dma_start(out=eb2s[:], in_=d["eb2"][:])
        nc.scalar.dma_start(out=pbTs[:], in_=d["pbT"][:])
        wqkh = [wqht[:, k] for k in range(KC)]
        wqkl = [wqlt[:, k] for k in range(KC)]
        wv = [wvt[:, k] for k in range(KC)]
        wp = [wpt[:, k] for k in range(KC)]

        chainq = []   # (tci, fc) entries awaiting chain emission
        pvq = []      # (b, fc, tt2, b2) awaiting PV emission

        def emit_fc(ent):
            tciP, fc = ent
            # all 4 chains (2 batches x 2 heads) of this fc, one transpose pair
            p1 = ppt.tile([128, 4, 256], F16, tag="p1")
            p2 = ppt.tile([80, 4, 256], F16, tag="p2")
            tt2 = ptt.tile([128, 8, 208], F16, tag="tt2")
            for b2 in range(2):
                b = tciP * 2 + b2
                bs = slice(b * N, (b + 1) * N)
                col0 = b * 12 + fc * 2
                es = []
                for j in range(2):
                    h = 2 * fc + j
                    col = col0 + j
                    scl = (sqkA if j == 0 else sqkB)[
                        :, tciP * 12 + fc * 2 + b2: tciP * 12 + fc * 2 + b2 + 1]
                    qrows = slice(64 * j, 64 * (j + 1))
                    s12 = pps2.tile([128, TCH], F32, tag="s12")
                    nc.tensor.matmul(
                        out=s12[:, 0:N], lhsT=sq[fc][qrows, b * N: b * N + 128],
                        rhs=sk[fc][qrows, bs], start=True, stop=True)
                    nc.tensor.matmul(
                        out=s12[0:69, N:2 * N],
                        lhsT=sq[fc][qrows, b * N + 128: (b + 1) * N],
                        rhs=sk[fc][qrows, bs], start=True, stop=True)
                    # merged exp over both halves ([69:,N:] is junk, unread)
                    f = pwork.tile([128, TCH], F32, tag="f")
                    nc.scalar.activation(f[:], s12[:], AF.Exp, scale=scl)
                    e1 = pe1.tile([128, N], F32, tag="e1")
                    e2 = pe1.tile([69, N], F32, tag="e2")
                    nc.vector.scalar_tensor_tensor(
                        out=e1[:], in0=f[:, 0:N], scalar=1.0, in1=eb1s[:, h],
                        op0=OP.mult, op1=OP.mult,
                        accum_out=rAB[:, 0, col:col + 1])
                    nc.vector.scalar_tensor_tensor(
                        out=e2[:], in0=f[0:69, N:2 * N], scalar=1.0,
                        in1=eb2s[:, h], op0=OP.mult, op1=OP.mult,
                        accum_out=rAB[0:69, 1, col:col + 1])
                    es.append((e1, e2))
                for j in range(2):
                    nc.vector.reciprocal(rrAB[:, :, col0 + j],
                                         rAB[:, :, col0 + j])
                    nc.vector.tensor_scalar(
                        out=rrS[:, :, col0 + j], in0=rrAB[:, :, col0 + j],
                        scalar1=255.0, scalar2=None, op0=OP.mult)
                # P-round: fp16 convert of e*rr + 1024 is RNE to int grid
                for j in range(2):
                    col = col0 + j
                    pj = b2 * 2 + j
                    e1, e2 = es[j]
                    nc.vector.tensor_scalar(
                        out=p1[:, pj, 0:N], in0=e1[:],
                        scalar1=rrS[:, 0, col:col + 1], scalar2=MP,
                        op0=OP.mult, op1=OP.add)
                    nc.vector.tensor_scalar(
                        out=p2[0:69, pj, 0:N], in0=e2[:],
                        scalar1=rrS[0:69, 1, col:col + 1], scalar2=MP,
                        op0=OP.mult, op1=OP.add)
            nc.sync.dma_start_transpose(tt2[:, :, 0:128], p1[:])
            nc.sync.dma_start_transpose(tt2[:, :, 128:208], p2[:])
            if os.environ.get("DUMP_OT") and not hasattr(emit_fc, "dumped"):
                emit_fc.dumped = (tciP, fc)
                nc.sync.dma_start(out=d["p1d"][:], in_=p1[:])
                nc.sync.dma_start(out=d["p2d"][:], in_=p2[:])
                nc.sync.dma_start(out=d["ttd"][:], in_=tt2[:])
            pvq.append((tciP * 2, fc, tt2, 0))
            pvq.append((tciP * 2 + 1, fc, tt2, 1))

        def emit_pv(item):
            b, fc, tt2, b2 = item
            opsum = ppo.tile([128, N], F32, tag="opsum")
            for j in range(2):
                hs = slice((2 * fc + j) * D, (2 * fc + j + 1) * D)
                orows = slice(64 * j, 64 * (j + 1))
                pj = (b2 * 2 + j) * 2
                nc.tensor.matmul(
                    out=opsum[orows, :], lhsT=vint[b][0][:, hs],
                    rhs=tt2[:, pj, 0:N], start=True, stop=False)
                nc.tensor.matmul(
                    out=opsum[orows, :], lhsT=vint[b][1][0:69, hs],
                    rhs=tt2[0:69, pj + 1, 0:N], start=False, stop=True)
            nc.scalar.activation(ot[fc][:, b * N:(b + 1) * N], opsum[:],
                                 AF.Identity, scale=CPV,
                                 bias=vsb[:, b, fc:fc + 1])

        def emit_proj(tciP):
            ts = slice(tciP * TCH, (tciP + 1) * TCH)
            for mf in range(KC):
                mfs = slice(mf * 128, (mf + 1) * 128)
                psy = ppy.tile([128, TCH], F32, tag="y")
                for k in range(KC):
                    nc.tensor.matmul(
                        out=psy[:], lhsT=wp[k][:, mfs], rhs=ot[k][:, ts],
                        start=(k == 0), stop=(k == KC - 1))
                ys = pvt.tile([128, TCH], F32, tag="ys")
                nc.scalar.activation(ys[:], psy[:], AF.Identity,
                                     bias=pbTs[:, mf:mf + 1], scale=1.0)
                nc.sync.dma_start(out=d["yt"][mfs, ts], in_=ys[:])

        for tci in range(T // TCH):
            ts = slice(tci * TCH, (tci + 1) * TCH)
            xhc = pxs.tile([128, KC, TCH], F16, tag="xhc")
            xlc = pxs.tile([128, KC, TCH], F16, tag="xlc")
            for k in range(KC):
                nc.sync.dma_start(out=xhc[:, k], in_=d["xh"][k][:, ts])
            for k in range(KC):
                nc.sync.dma_start(out=xlc[:, k], in_=d["xl"][k][:, ts])

            def emit_v():
            # ---- V projection + quantize (fp16 magic-round, offset clip)
                for b2 in range(2):
                    b = tci * 2 + b2
                    for mi, (m0, m1) in enumerate([(0, 128), (128, N)]):
                        msz = m1 - m0
                        R = pvt.tile([128, C], F16, tag="R")
                        for n0, n1 in [(0, 512), (512, C)]:
                            psv = ppv.tile([128, 512], F32, tag="v")
                            for k in range(KC):
                                nc.tensor.matmul(
                                    out=psv[0:msz, 0:n1 - n0],
                                    lhsT=xhc[:, k, b2 * N + m0: b2 * N + m1],
                                    rhs=wv[k][:, n0:n1],
                                    start=(k == 0), stop=(k == KC - 1))
                            nc.vector.tensor_scalar(
                                out=R[0:msz, n0:n1], in0=psv[0:msz, 0:n1 - n0],
                                scalar1=63.5, scalar2=MV, op0=OP.mult, op1=OP.add)
                        vc = pvt.tile([128, C], F16, tag="vc")
                        nc.vector.tensor_scalar(
                            out=vc[0:msz, :], in0=R[0:msz, :],
                            scalar1=MV - 127.0, scalar2=MV + 127.0,
                            op0=OP.max, op1=OP.min)
                        nc.scalar.activation(vint[b][mi][0:msz, :], vc[0:msz, :],
                                             AF.Identity, bias=bmMV[0:msz, :],
                                             scale=1.0)
                    # V column sums for the P-offset correction (tiny PE matmuls)
                    vs = ppv.tile([128, 512], F32, tag="v", name="vs")
                    for mf in range(KC):
                        for blk, msz in [(0, 128), (1, 69)]:
                            nc.tensor.matmul(
                                out=vs[0:128, mf:mf + 1],
                                lhsT=vint[b][blk][0:msz, mf * 128:(mf + 1) * 128],
                                rhs=onesc[0:msz, :], start=(blk == 0), stop=(blk == 1))
                    nc.vector.tensor_scalar(
                        out=vsb[:, b, :], in0=vs[0:128, 0:KC],
                        scalar1=CORR, scalar2=None, op0=OP.mult)

            last = tci == T // TCH - 1
            if last:
                emit_v()   # last chunk: PVs must drain during its QK loop

            # ---- Q/K projection, 3-pass fp16; drain prev-chunk chains
            passes = [(wqkh, xhc), (wqkl, xhc), (wqkh, xlc)]
            # last chunk: pair q/k blocks (fc, 6+fc) so each fc's chains can
            # start right away and overlap the remaining QK matmuls
            fcporder = ([x for fc in range(6) for x in (fc, fc + 6)]
                        if tci == T // TCH - 1 else list(range(12)))
            for fci, fcp in enumerate(fcporder):
                fs = slice(fcp * 128, (fcp + 1) * 128)
                ps = pqk.tile([128, TCH], F32, tag="qk")
                i = 0
                for wt, xt in passes:
                    for k in range(KC):
                        nc.tensor.matmul(
                            out=ps[:], lhsT=wt[k][:, fs], rhs=xt[:, k],
                            start=(i == 0), stop=(i == 3 * KC - 1))
                        i += 1
                dst = sq[fcp] if fcp < 6 else sk[fcp - 6]
                nc.scalar.activation(dst[:, ts], ps[:], AF.Sign)
                nc.vector.tensor_reduce(
                    out=pa[:, tci * 24 + fcp * 2: tci * 24 + fcp * 2 + 2],
                    in_=ps[:].rearrange("p (a b) -> p a b", a=2),
                    axis=mybir.AxisListType.X, op=OP.add,
                    apply_absolute_value=True)
                if chainq and (last or fci % 2 == 1):
                    emit_fc(chainq.pop(0))
                while len(pvq) > 4 and (last or pvq[0][0] < tci * 2):
                    emit_pv(pvq.pop(0))
                if last and fcp >= 6:
                    # per-fc scales for the final chunk so its chains start
                    # now and overlap the remaining QK work (no next chunk)
                    fc = fcp - 6
                    pav = pa[:, tci * 24:(tci + 1) * 24].rearrange(
                        "p (h f) -> p h f", h=2)[:, :, fc * 2:fc * 2 + 2]
                    sA4 = ppv.tile([128, 512], F32, tag="v", name="sA4")
                    sB4 = ppv.tile([128, 512], F32, tag="v", name="sB4")
                    nc.tensor.matmul(out=sA4[0:128, 0:4], lhsT=ind2[:, 0:128],
                                     rhs=pav, start=True, stop=True)
                    nc.tensor.matmul(out=sB4[0:128, 0:4], lhsT=ind2[:, 128:256],
                                     rhs=pav, start=True, stop=True)
                    sAs4 = pvt.tile([128, 4], F32, tag="sAs4")
                    sBs4 = pvt.tile([128, 4], F32, tag="sBs4")
                    nc.vector.tensor_copy(sAs4[:], sA4[0:128, 0:4])
                    nc.vector.tensor_copy(sBs4[:], sB4[0:128, 0:4])
                    sc4 = slice(tci * 12 + fc * 2, tci * 12 + fc * 2 + 2)
                    nc.vector.scalar_tensor_tensor(
                        out=sqkA[:, sc4], in0=sAs4[:, 0:2], scalar=SCALE,
                        in1=sAs4[:, 2:4], op0=OP.mult, op1=OP.mult)
                    nc.vector.scalar_tensor_tensor(
                        out=sqkB[:, sc4], in0=sBs4[:, 0:2], scalar=SCALE,
                        in1=sBs4[:, 2:4], op0=OP.mult, op1=OP.mult)
                    emit_fc((tci, fc))

            if not last:
                emit_v()

            if tci < T // TCH - 1:
                # ---- batched scales for this chunk
                pc = slice(tci * 24, tci * 24 + 24)
                sA = ppv.tile([128, 512], F32, tag="v", name="sA")
                sB = ppv.tile([128, 512], F32, tag="v", name="sB")
                nc.tensor.matmul(out=sA[0:128, 0:24], lhsT=ind2[:, 0:128],
                                 rhs=pa[:, pc], start=True, stop=True)
                nc.tensor.matmul(out=sB[0:128, 0:24], lhsT=ind2[:, 128:256],
                                 rhs=pa[:, pc], start=True, stop=True)
                sAs = pvt.tile([128, 24], F32, tag="sAs")
                sBs = pvt.tile([128, 24], F32, tag="sBs")
                nc.vector.tensor_copy(sAs[:], sA[0:128, 0:24])
                nc.vector.tensor_copy(sBs[:], sB[0:128, 0:24])
                sc = slice(tci * 12, tci * 12 + 12)
                nc.vector.scalar_tensor_tensor(
                    out=sqkA[:, sc], in0=sAs[:, 0:12], scalar=SCALE,
                    in1=sAs[:, 12:24], op0=OP.mult, op1=OP.mult)
                nc.vector.scalar_tensor_tensor(
                    out=sqkB[:, sc], in0=sBs[:, 0:12], scalar=SCALE,
                    in1=sBs[:, 12:24], op0=OP.mult, op1=OP.mult)

            # ---- drain pending chains/PVs of the previous chunk, then proj
            while chainq:
                emit_fc(chainq.pop(0))
            while pvq and pvq[0][0] < tci * 2:
                emit_pv(pvq.pop(0))
            if tci > 0:
                emit_proj(tci - 1)
            if tci < T // TCH - 1:
                chainq = [(tci, fc) for fc in range(6)]

        while chainq:
            emit_fc(chainq.pop(0))
        while pvq:
            emit_pv(pvq.pop(0))
        emit_proj(T // TCH - 1)

        if os.environ.get("DUMP_OT"):
            for k in range(KC):
                nc.sync.dma_start(out=d["otd"][k], in_=ot[k][:])
            for b in range(BL):
                nc.sync.dma_start(out=d["vd"][b], in_=vint[b][0][:])
            nc.sync.dma_start(out=d["rd"][:], in_=rAB[:])

    nc.compile()
    return nc


def _prep_host(x, qkv_w, proj_w, proj_b, rel_table, rel_index):
    def f16(a):
        return np.ascontiguousarray(a, dtype=np.float16)

    def chunkT(a):  # [rows, cols] -> Cin-chunked [KC, 128, cols]
        return np.ascontiguousarray(a.reshape(KC, 128, -1))

    wqk = qkv_w[:2 * C].T.astype(np.float32)          # [768, 1536]
    wqkh = wqk.astype(np.float16)
    wqkl = (wqk - wqkh.astype(np.float32)).astype(np.float16)
    wv = qkv_w[2 * C:].T.astype(np.float16)           # [768, 768]
    wp = proj_w.T.astype(np.float16)
    pbT = np.ascontiguousarray(proj_b.astype(np.float32).reshape(KC, 128).T)
    eb = np.exp(rel_table[rel_index].astype(np.float32))   # [N, N, H]
    eb = np.ascontiguousarray(eb.transpose(0, 2, 1))       # [n, h, m]
    ind2 = np.zeros((128, 256), np.float32)
    ind2[0:64, 0:128] = 1.0 / (N * D)
    ind2[64:128, 128:256] = 1.0 / (N * D)

    shared = {
        "wqkh": chunkT(f16(wqkh)), "wqkl": chunkT(f16(wqkl)),
        "wv": chunkT(f16(wv)), "wp": chunkT(f16(wp)),
        "pbT": pbT, "eb1": eb[0:128], "eb2": np.ascontiguousarray(eb[128:N]),
        "ind2": ind2,
    }
    in_maps = []
    for c in range(NCORES):
        xc = x[c * BL:(c + 1) * BL].reshape(T, C).T.astype(np.float32)
        xh = xc.astype(np.float16)
        xlo = (xc - xh.astype(np.float32)).astype(np.float16)
        in_maps.append({"xh": chunkT(xh), "xl": chunkT(xlo), **shared})
    return in_maps


def kernel(x, qkv_w, proj_w, proj_b, rel_table, rel_index):
    x = np.asarray(x, np.float32)
    qkv_w = np.asarray(qkv_w, np.float32)
    proj_w = np.asarray(proj_w, np.float32)
    proj_b = np.asarray(proj_b, np.float32)
    rel_table = np.asarray(rel_table, np.float32)
    rel_index = np.asarray(rel_index, np.int32)

    if "nc" not in _CACHE:
        _CACHE["nc"] = _build()
    nc = _CACHE["nc"]
    in_maps = _prep_host(x, qkv_w, proj_w, proj_b, rel_table, rel_index)
    res = run_bass_kernel_spmd(nc, in_maps, list(range(NCORES)))
    out = np.empty((B, N, C), np.float32)
    for c in range(NCORES):
        yt = res.results[c]["yt"]                      # [C, T]
        out[c * BL:(c + 1) * BL] = yt.T.reshape(BL, N, C)
    return out


# revision 24
# speedup vs baseline: 1.0529x; 1.0529x over previous
"""Trainium2 Bass kernel for nn_BinaryAttention (dense transformer block).

Strategy: data-parallel over batch, 8 cores x 8 batch elements each.
v2: zero-gpsimd pipeline. Per core (4 chunks of 2 batches):
  QKV proj: Q/K in 3-pass fp16 split precision (sign() boundaries need
    ~22-bit products); V 1-pass fp16. ACT Sign -> fp8 sq/sk; DVE
    abs-reduce -> per-(b,h) scale partials.
  scales: batched per chunk via indicator matmul + one STT.
  V quantize: DVE TS (x*63.5 + 1536 -> fp16, RNE int round), DVE
    offset-clip [1409,1663], ACT Identity -1536 -> exact int8 grid fp16.
  attention (per (b,h) pair): S = sq@sk^T exact fp8 matmul; one merged
    ACT Exp [128,394] (scale=s_q*s_k/8 per-partition AP); DVE STT
    e = f*expb with fused rowsum accum; batched recip+*255; P-round in
    ONE op per half: fp16(e*rr + 1024) = 1024 + round(255*p) (RNE via
    fp16 convert); DMA-xbar transpose (both heads of an fc in one
    instruction); PV matmul in fp16 carries the +1024 offset, corrected
    exactly via per-partition ACT bias (-1024*CPV*colsum(V), colsums by
    tiny PE matmuls).
  proj: 1-pass fp16, bias added via ACT Identity on the PSUM->SBUF copy.
Chains software-pipelined one chunk behind their QK projection so
ACT/DVE chew on chunk i while PE runs chunk i+1 matmuls.
"""
import sys
sys.path.insert(0, "/opt/trn_rl_repo")
import os
import numpy as np
from contextlib import ExitStack

import concourse.bass as bass
import concourse.tile as tile
from concourse import bacc, mybir
from concourse.bass_utils import run_bass_kernel_spmd

F32 = mybir.dt.float32
F16 = mybir.dt.float16
BF16 = mybir.dt.bfloat16
F8 = mybir.dt.float8e4
AF = mybir.ActivationFunctionType
OP = mybir.AluOpType

NCORES = 8
B, N, C = 64, 197, 768
H, D = 12, 64
KC = C // 128               # 6 contraction chunks
BL = B // NCORES            # 8 batches per core
T = BL * N                  # 1576 tokens per core
TCH = 2 * N                 # 394-token chunks (2 batches), 4 per core
SCALE = float(D) ** -0.5
CPV = float(np.float32(1.0 / 255.0) * np.float32(2.0 / 127.0))
MP = 1024.0                 # P-round offset: p+MP in [1024,2048) = fp16 int grid
MV = 1536.0                 # V-round offset: v*63.5+MV in [1409,1663]
CORR = float(-MP * CPV)

_CACHE = {}


def _build():
    nc = bacc.Bacc(None, target_bir_lowering=False, debug=False)
    d = {}

    def din(name, shape, dt):
        d[name] = nc.dram_tensor(name, list(shape), dt, kind="ExternalInput")

    din("xh", (KC, 128, T), F16)
    din("xl", (KC, 128, T), F16)
    din("wqkh", (KC, 128, 2 * C), F16)
    din("wqkl", (KC, 128, 2 * C), F16)
    din("wv", (KC, 128, C), F16)
    din("wp", (KC, 128, C), F16)
    din("pbT", (128, KC), F32)          # proj bias, feature-partition layout
    din("eb1", (128, H, N), F32)        # exp(bias) rows n=0..127   [n, h, m]
    din("eb2", (69, H, N), F32)         # exp(bias) rows n=128..196
    din("ind2", (128, 256), F32)
    d["yt"] = nc.dram_tensor("yt", [C, T], F32, kind="ExternalOutput")
    DUMP = os.environ.get("DUMP_OT")
    if DUMP:
        d["otd"] = nc.dram_tensor("otd", [KC, 128, T], F16, kind="ExternalOutput")
        d["p1d"] = nc.dram_tensor("p1d", [128, 4, 256], F16, kind="ExternalOutput")
        d["p2d"] = nc.dram_tensor("p2d", [80, 4, 256], F16, kind="ExternalOutput")
        d["ttd"] = nc.dram_tensor("ttd", [128, 8, 208], F16, kind="ExternalOutput")
        d["vd"] = nc.dram_tensor("vd", [BL, 128, C], F16, kind="ExternalOutput")
        d["rd"] = nc.dram_tensor("rd", [128, 2, 96], F32, kind="ExternalOutput")

    with tile.TileContext(nc) as tc, ExitStack() as ctx:
        pers = ctx.enter_context(tc.tile_pool(name="pers", bufs=1))
        pw = ctx.enter_context(tc.tile_pool(name="pw", bufs=1))
        pxs = ctx.enter_context(tc.tile_pool(name="pxs", bufs=2))
        pvt = ctx.enter_context(tc.tile_pool(name="pvt", bufs=2))
        pwork = ctx.enter_context(tc.tile_pool(name="pwork", bufs=3))
        pe1 = ctx.enter_context(tc.tile_pool(name="pe1", bufs=6))
        ppt = ctx.enter_context(tc.tile_pool(name="ppt", bufs=2))
        ptt = ctx.enter_context(tc.tile_pool(name="ptt", bufs=4))
        # PSUM: exactly 8 banks
        pqk = ctx.enter_context(tc.tile_pool(name="pqk", bufs=2, space="PSUM"))
        ppv = ctx.enter_context(tc.tile_pool(name="ppv", bufs=2, space="PSUM"))
        pps2 = ctx.enter_context(tc.tile_pool(name="pps2", bufs=2, space="PSUM"))
        ppo = ctx.enter_context(tc.tile_pool(name="ppo", bufs=1, space="PSUM"))
        ppy = ctx.enter_context(tc.tile_pool(name="ppy", bufs=1, space="PSUM"))

        # ---- persistent state / constants (DMAs issued after the weight
        # loads on the scalar queue; sync queue stays free for x chunk 0)
        ind2 = pers.tile([128, 256], F32, tag="ind2")
        eb1s = pers.tile([128, H, N], F32, tag="eb1s")
        eb2s = pers.tile([69, H, N], F32, tag="eb2s")
        pbTs = pers.tile([128, KC], F32, tag="pbTs")

        onesc = pers.tile([128, 1], F16, tag="onesc")
        nc.vector.memset(onesc[:], 1.0)
        bMP = pers.tile([128, 1], F32, tag="bMP")
        nc.vector.memset(bMP[:], MP)
        bmMV = pers.tile([128, 1], F32, tag="bmMV")
        nc.vector.memset(bmMV[:], -MV)

        sq = [pers.tile([128, T], F8, tag=f"sq{i}", name=f"sq{i}") for i in range(KC)]
        sk = [pers.tile([128, T], F8, tag=f"sk{i}", name=f"sk{i}") for i in range(KC)]
        vint = [(pers.tile([128, C], F16, tag=f"v1_{b}", name=f"v1_{b}"),
                 pers.tile([69, C], F16, tag=f"v2_{b}", name=f"v2_{b}")) for b in range(BL)]
        ot = [pers.tile([128, T], F16, tag=f"ot{i}", name=f"ot{i}") for i in range(KC)]
        pa = pers.tile([128, 96], F32, tag="pa")
        sqkA = pers.tile([128, 48], F32, tag="sqkA")   # col = tci*12+fc*2+b2
        sqkB = pers.tile([128, 48], F32, tag="sqkB")
        rAB = pers.tile([128, 2, 96], F32, tag="rAB")  # col = b*12+fc*2+j
        rrAB = pers.tile([128, 2, 96], F32, tag="rrAB")
        rrS = pers.tile([128, 2, 96], F32, tag="rrS")
        vsb = pers.tile([128, BL, KC], F32, tag="vsb")

        # ---- weights resident
        wqht = pw.tile([128, KC, 2 * C], F16, tag="wqht")
        wqlt = pw.tile([128, KC, 2 * C], F16, tag="wqlt")
        wvt = pw.tile([128, KC, C], F16, tag="wvt")
        wpt = pw.tile([128, KC, C], F16, tag="wpt")
        # per-k loads in QK-use order so fcp0 starts after the first slices
        for k in range(KC):
            nc.scalar.dma_start(out=wqht[:, k], in_=d["wqkh"][k])
        for k in range(KC):
            nc.scalar.dma_start(out=wqlt[:, k], in_=d["wqkl"][k])
        for k in range(KC):
            nc.scalar.dma_start(out=wvt[:, k], in_=d["wv"][k])
        for k in range(KC):
            nc.scalar.dma_start(out=wpt[:, k], in_=d["wp"][k])
        nc.scalar.dma_start(out=ind2[:], in_=d["ind2"][:])
        nc.scalar.dma_start(out=eb1s[:], in_=d["eb1"][:])
        nc.scalar.dma_start(out=eb2s[:], in_=d["eb2"][:])
        nc.scalar.dma_start(out=pbTs[:], in_=d["pbT"][:])
        wqkh = [wqht[:, k] for k in range(KC)]
        wqkl = [wqlt[:, k] for k in range(KC)]
        wv = [wvt[:, k] for k in range(KC)]
        wp = [wpt[:, k] for k in range(KC)]

        chainq = []   # (tci, fc) entries awaiting chain emission
        pvq = []      # (b, fc, tt2, b2) awaiting PV emission

        def emit_fc(ent):
            tciP, fc = ent
            # all 4 chains (2 batches x 2 heads) of this fc, one transpose pair
            p1 = ppt.tile([128, 4, 256], F16, tag="p1")
            p2 = ppt.tile([80, 4, 256], F16, tag="p2")
            tt2 = ptt.tile([128, 8, 208], F16, tag="tt2")
            for b2 in range(2):
                b = tciP * 2 + b2
                bs = slice(b * N, (b + 1) * N)
                col0 = b * 12 + fc * 2
                es = []
                for j in range(2):
                    h = 2 * fc + j
                    col = col0 + j
                    scl = (sqkA if j == 0 else sqkB)[
                        :, tciP * 12 + fc * 2 + b2: tciP * 12 + fc * 2 + b2 + 1]
                    qrows = slice(64 * j, 64 * (j + 1))
                    s12 = pps2.tile([128, TCH], F32, tag="s12")
                    nc.tensor.matmul(
                        out=s12[:, 0:N], lhsT=sq[fc][qrows, b * N: b * N + 128],
                        rhs=sk[fc][qrows, bs], start=True, stop=True)
                    nc.tensor.matmul(
                        out=s12[0:69, N:2 * N],
                        lhsT=sq[fc][qrows, b * N + 128: (b + 1) * N],
                        rhs=sk[fc][qrows, bs], start=True, stop=True)
                    # merged exp over both halves ([69:,N:] is junk, unread)
                    f = pwork.tile([128, TCH], F32, tag="f")
                    nc.scalar.activation(f[:], s12[:], AF.Exp, scale=scl)
                    e1 = pe1.tile([128, N], F32, tag="e1")
                    e2 = pe1.tile([69, N], F32, tag="e2")
                    nc.vector.scalar_tensor_tensor(
                        out=e1[:], in0=f[:, 0:N], scalar=1.0, in1=eb1s[:, h],
                        op0=OP.mult, op1=OP.mult,
                        accum_out=rAB[:, 0, col:col + 1])
                    nc.vector.scalar_tensor_tensor(
                        out=e2[:], in0=f[0:69, N:2 * N], scalar=1.0,
                        in1=eb2s[:, h], op0=OP.mult, op1=OP.mult,
                        accum_out=rAB[0:69, 1, col:col + 1])
                    es.append((e1, e2))
                for j in range(2):
                    nc.vector.reciprocal(rrAB[:, :, col0 + j],
                                         rAB[:, :, col0 + j])
                    nc.vector.tensor_scalar(
                        out=rrS[:, :, col0 + j], in0=rrAB[:, :, col0 + j],
                        scalar1=255.0, scalar2=None, op0=OP.mult)
                # P-round: fp16 convert of e*rr + 1024 is RNE to int grid
                for j in range(2):
                    col = col0 + j
                    pj = b2 * 2 + j
                    e1, e2 = es[j]
                    nc.vector.tensor_scalar(
                        out=p1[:, pj, 0:N], in0=e1[:],
                        scalar1=rrS[:, 0, col:col + 1], scalar2=MP,
                        op0=OP.mult, op1=OP.add)
                    nc.vector.tensor_scalar(
                        out=p2[0:69, pj, 0:N], in0=e2[:],
                        scalar1=rrS[0:69, 1, col:col + 1], scalar2=MP,
                        op0=OP.mult, op1=OP.add)
            nc.sync.dma_start_transpose(tt2[:, :, 0:128], p1[:])
            nc.sync.dma_start_transpose(tt2[:, :, 128:208], p2[:])
            if os.environ.get("DUMP_OT") and not hasattr(emit_fc, "dumped"):
                emit_fc.dumped = (tciP, fc)
                nc.sync.dma_start(out=d["p1d"][:], in_=p1[:])
                nc.sync.dma_start(out=d["p2d"][:], in_=p2[:])
                nc.sync.dma_start(out=d["ttd"][:], in_=tt2[:])
            pvq.append((tciP * 2, fc, tt2, 0))
            pvq.append((tciP * 2 + 1, fc, tt2, 1))

        def emit_pv(item):
            b, fc, tt2, b2 = item
            opsum = ppo.tile([128, N], F32, tag="opsum")
            for j in range(2):
                hs = slice((2 * fc + j) * D, (2 * fc + j + 1) * D)
                orows = slice(64 * j, 64 * (j + 1))
                pj = (b2 * 2 + j) * 2
                nc.tensor.matmul(
                    out=opsum[orows, :], lhsT=vint[b][0][:, hs],
                    rhs=tt2[:, pj, 0:N], start=True, stop=False)
                nc.tensor.matmul(
                    out=opsum[orows, :], lhsT=vint[b][1][0:69, hs],
                    rhs=tt2[0:69, pj + 1, 0:N], start=False, stop=True)
            nc.scalar.activation(ot[fc][:, b * N:(b + 1) * N], opsum[:],
                                 AF.Identity, scale=CPV,
                                 bias=vsb[:, b, fc:fc + 1])

        def emit_proj(tciP):
            ts = slice(tciP * TCH, (tciP + 1) * TCH)
            for mf in range(KC):
                mfs = slice(mf * 128, (mf + 1) * 128)
                psy = ppy.tile([128, TCH], F32, tag="y")
                for k in range(KC):
                    nc.tensor.matmul(
                        out=psy[:], lhsT=wp[k][:, mfs], rhs=ot[k][:, ts],
                        start=(k == 0), stop=(k == KC - 1))
                ys = pvt.tile([128, TCH], F32, tag="ys")
                nc.scalar.activation(ys[:], psy[:], AF.Identity,
                                     bias=pbTs[:, mf:mf + 1], scale=1.0)
                nc.sync.dma_start(out=d["yt"][mfs, ts], in_=ys[:])

        for tci in range(T // TCH):
            ts = slice(tci * TCH, (tci + 1) * TCH)
            xhc = pxs.tile([128, KC, TCH], F16, tag="xhc")
            xlc = pxs.tile([128, KC, TCH], F16, tag="xlc")
            nc.sync.dma_start(out=xhc[:], in_=d["xh"][:, :, ts].rearrange("k p f -> p k f"))
            nc.sync.dma_start(out=xlc[:], in_=d["xl"][:, :, ts].rearrange("k p f -> p k f"))

            def emit_v():
            # ---- V projection + quantize (fp16 magic-round, offset clip)
                for b2 in range(2):
                    b = tci * 2 + b2
                    for mi, (m0, m1) in enumerate([(0, 128), (128, N)]):
                        msz = m1 - m0
                        R = pvt.tile([128, C], F16, tag="R")
                        for n0, n1 in [(0, 512), (512, C)]:
                            psv = ppv.tile([128, 512], F32, tag="v")
                            for k in range(KC):
                                nc.tensor.matmul(
                                    out=psv[0:msz, 0:n1 - n0],
                                    lhsT=xhc[:, k, b2 * N + m0: b2 * N + m1],
                                    rhs=wv[k][:, n0:n1],
                                    start=(k == 0), stop=(k == KC - 1))
                            nc.vector.tensor_scalar(
                                out=R[0:msz, n0:n1], in0=psv[0:msz, 0:n1 - n0],
                                scalar1=63.5, scalar2=MV, op0=OP.mult, op1=OP.add)
                        vc = pvt.tile([128, C], F16, tag="vc")
                        nc.vector.tensor_scalar(
                            out=vc[0:msz, :], in0=R[0:msz, :],
                            scalar1=MV - 127.0, scalar2=MV + 127.0,
                            op0=OP.max, op1=OP.min)
                        nc.scalar.activation(vint[b][mi][0:msz, :], vc[0:msz, :],
                                             AF.Identity, bias=bmMV[0:msz, :],
                                             scale=1.0)
                    # V column sums for the P-offset correction (tiny PE matmuls)
                    vs = ppv.tile([128, 512], F32, tag="v", name="vs")
                    for mf in range(KC):
                        for blk, msz in [(0, 128), (1, 69)]:
                            nc.tensor.matmul(
                                out=vs[0:128, mf:mf + 1],
                                lhsT=vint[b][blk][0:msz, mf * 128:(mf + 1) * 128],
                                rhs=onesc[0:msz, :], start=(blk == 0), stop=(blk == 1))
                    nc.vector.tensor_scalar(
                        out=vsb[:, b, :], in0=vs[0:128, 0:KC],
                        scalar1=CORR, scalar2=None, op0=OP.mult)

            last = tci == T // TCH - 1
            if last:
                emit_v()   # last chunk: PVs must drain during its QK loop

            # ---- Q/K projection, 3-pass fp16; drain prev-chunk chains
            passes = [(wqkh, xhc), (wqkl, xhc), (wqkh, xlc)]
            # last chunk: pair q/k blocks (fc, 6+fc) so each fc's chains can
            # start right away and overlap the remaining QK matmuls
            fcporder = ([x for fc in range(6) for x in (fc, fc + 6)]
                        if tci == T // TCH - 1 else list(range(12)))
            for fci, fcp in enumerate(fcporder):
                fs = slice(fcp * 128, (fcp + 1) * 128)
                ps = pqk.tile([128, TCH], F32, tag="qk")
                i = 0
                for wt, xt in passes:
                    for k in range(KC):
                        nc.tensor.matmul(
                            out=ps[:], lhsT=wt[k][:, fs], rhs=xt[:, k],
                            start=(i == 0), stop=(i == 3 * KC - 1))
                        i += 1
                dst = sq[fcp] if fcp < 6 else sk[fcp - 6]
                nc.scalar.activation(dst[:, ts], ps[:], AF.Sign)
                nc.vector.tensor_reduce(
                    out=pa[:, tci * 24 + fcp * 2: tci * 24 + fcp * 2 + 2],
                    in_=ps[:].rearrange("p (a b) -> p a b", a=2),
                    axis=mybir.AxisListType.X, op=OP.add,
                    apply_absolute_value=True)
                if chainq and (last or fci % 2 == 1):
                    emit_fc(chainq.pop(0))
                while len(pvq) > 4 and (last or pvq[0][0] < tci * 2):
                    emit_pv(pvq.pop(0))
                if last and fcp >= 6:
                    # per-fc scales for the final chunk so its chains start
                    # now and overlap the remaining QK work (no next chunk)
                    fc = fcp - 6
                    pav = pa[:, tci * 24:(tci + 1) * 24].rearrange(
                        "p (h f) -> p h f", h=2)[:, :, fc * 2:fc * 2 + 2]
                    sA4 = ppv.tile([128, 512], F32, tag="v", name="sA4")
                    sB4 = ppv.tile([128, 512], F32, tag="v", name="sB4")
                    nc.tensor.matmul(out=sA4[0:128, 0:4], lhsT=ind2[:, 0:128],
                                     rhs=pav, start=True, stop=True)
                    nc.tensor.matmul(out=sB4[0:128, 0:4], lhsT=ind2[:, 128:256],
                                     rhs=pav, start=True, stop=True)
                    sAs4 = pvt.tile([128, 4], F32, tag="sAs4")
                    sBs4 = pvt.tile([128, 4], F32, tag="sBs4")
                    nc.vector.tensor_copy(sAs4[:], sA4[0:128, 0:4])
                    nc.vector.tensor_copy(sBs4[:], sB4[0:128, 0:4])
                    sc4 = slice(tci * 12 + fc * 2, tci * 12 + fc * 2 + 2)
                    nc.vector.scalar_tensor_tensor(
                        out=sqkA[:, sc4], in0=sAs4[:, 0:2], scalar=SCALE,
                        in1=sAs4[:, 2:4], op0=OP.mult, op1=OP.mult)
                    nc.vector.scalar_tensor_tensor(
                        out=sqkB[:, sc4], in0=sBs4[:, 0:2], scalar=SCALE,
                        in1=sBs4[:, 2:4], op0=OP.mult, op1=OP.mult)
                    emit_fc((tci, fc))

            if not last:
                emit_v()

            if tci < T // TCH - 1:
                # ---- batched scales for this chunk
                pc = slice(tci * 24, tci * 24 + 24)
                sA = ppv.tile([128, 512], F32, tag="v", name="sA")
                sB = ppv.tile([128, 512], F32, tag="v", name="sB")
                nc.tensor.matmul(out=sA[0:128, 0:24], lhsT=ind2[:, 0:128],
                                 rhs=pa[:, pc], start=True, stop=True)
                nc.tensor.matmul(out=sB[0:128, 0:24], lhsT=ind2[:, 128:256],
                                 rhs=pa[:, pc], start=True, stop=True)
                sAs = pvt.tile([128, 24], F32, tag="sAs")
                sBs = pvt.tile([128, 24], F32, tag="sBs")
                nc.vector.tensor_copy(sAs[:], sA[0:128, 0:24])
                nc.vector.tensor_copy(sBs[:], sB[0:128, 0:24])
                sc = slice(tci * 12, tci * 12 + 12)
                nc.vector.scalar_tensor_tensor(
                    out=sqkA[:, sc], in0=sAs[:, 0:12], scalar=SCALE,
                    in1=sAs[:, 12:24], op0=OP.mult, op1=OP.mult)
                nc.vector.scalar_tensor_tensor(
                    out=sqkB[:, sc], in0=sBs[:, 0:12], scalar=SCALE,
                    in1=sBs[:, 12:24], op0=OP.mult, op1=OP.mult)

            # ---- drain pending chains/PVs of the previous chunk, then proj
            while chainq:
                emit_fc(chainq.pop(0))
            while pvq and pvq[0][0] < tci * 2:
                emit_pv(pvq.pop(0))
            if tci > 0:
                emit_proj(tci - 1)
            if tci < T // TCH - 1:
                chainq = [(tci, fc) for fc in range(6)]

        while chainq:
            emit_fc(chainq.pop(0))
        while pvq:
            emit_pv(pvq.pop(0))
        emit_proj(T // TCH - 1)

        if os.environ.get("DUMP_OT"):
            for k in range(KC):
                nc.sync.dma_start(out=d["otd"][k], in_=ot[k][:])
            for b in range(BL):
                nc.sync.dma_start(out=d["vd"][b], in_=vint[b][0][:])
            nc.sync.dma_start(out=d["rd"][:], in_=rAB[:])

    nc.compile()
    return nc


def _prep_host(x, qkv_w, proj_w, proj_b, rel_table, rel_index):
    def f16(a):
        return np.ascontiguousarray(a, dtype=np.float16)

    def chunkT(a):  # [rows, cols] -> Cin-chunked [KC, 128, cols]
        return np.ascontiguousarray(a.reshape(KC, 128, -1))

    wqk = qkv_w[:2 * C].T.astype(np.float32)          # [768, 1536]
    wqkh = wqk.astype(np.float16)
    wqkl = (wqk - wqkh.astype(np.float32)).astype(np.float16)
    wv = qkv_w[2 * C:].T.astype(np.float16)           # [768, 768]
    wp = proj_w.T.astype(np.float16)
    pbT = np.ascontiguousarray(proj_b.astype(np.float32).reshape(KC, 128).T)
    eb = np.exp(rel_table[rel_index].astype(np.float32))   # [N, N, H]
    eb = np.ascontiguousarray(eb.transpose(0, 2, 1))       # [n, h, m]
    ind2 = np.zeros((128, 256), np.float32)
    ind2[0:64, 0:128] = 1.0 / (N * D)
    ind2[64:128, 128:256] = 1.0 / (N * D)

    shared = {
        "wqkh": chunkT(f16(wqkh)), "wqkl": chunkT(f16(wqkl)),
        "wv": chunkT(f16(wv)), "wp": chunkT(f16(wp)),
        "pbT": pbT, "eb1": eb[0:128], "eb2": np.ascontiguousarray(eb[128:N]),
        "ind2": ind2,
    }
    in_maps = []
    for c in range(NCORES):
        xc = x[c * BL:(c + 1) * BL].reshape(T, C).T.astype(np.float32)
        xh = xc.astype(np.float16)
        xlo = (xc - xh.astype(np.float32)).astype(np.float16)
        in_maps.append({"xh": chunkT(xh), "xl": chunkT(xlo), **shared})
    return in_maps


def kernel(x, qkv_w, proj_w, proj_b, rel_table, rel_index):
    x = np.asarray(x, np.float32)
    qkv_w = np.asarray(qkv_w, np.float32)
    proj_w = np.asarray(proj_w, np.float32)
    proj_b = np.asarray(proj_b, np.float32)
    rel_table = np.asarray(rel_table, np.float32)
    rel_index = np.asarray(rel_index, np.int32)

    if "nc" not in _CACHE:
        _CACHE["nc"] = _build()
    nc = _CACHE["nc"]
    in_maps = _prep_host(x, qkv_w, proj_w, proj_b, rel_table, rel_index)
    res = run_bass_kernel_spmd(nc, in_maps, list(range(NCORES)))
    out = np.empty((B, N, C), np.float32)
    for c in range(NCORES):
        yt = res.results[c]["yt"]                      # [C, T]
        out[c * BL:(c + 1) * BL] = yt.T.reshape(BL, N, C)
    return out
